# revision 1
# baseline (speedup 1.0000x reference)
"""Trainium2 Bass kernel for fused multi-head attention (B=4, N=2048, D=384, h=8, dh=48).

Sharding: 32 (batch, head) pairs across 8 cores -> core c handles batch c//2 and
heads [4*(c%2), 4*(c%2)+4). Each core computes a *partial* output projection
(its 4 heads' contribution to out @ Wproj); the host sums the two partials per
batch and adds bproj.

Per-core algorithm (everything in "transposed" layout so no PE transposes are
needed):
  xT   [384, 2048]  = x^T                          (transposed on host, bf16)
  QT   [256, 2048]  = (Wq_pad)^T @ xT  (4 heads padded dh 48->64, pair-packed)
  KT   same
  V'   [2048, 4*64] = x @ Wv_pad, with a ones-column per head at col h*64+32
                      (gpsimd memset) -> the PV matmul also accumulates the
                      softmax denominator Z for free.
  per (pair, q-half, head):
      simT[k,q] = KT_h^T @ QT_h     (PSUM, K=64)
      E = exp(simT)                 (ACT, psum->sbuf bf16; scores ~N(0,1) so no
                                     max subtraction is needed for stability)
      acc[o:o+64, q] += V'_kc^T @ E (PSUM accumulate; row o+32 = Z)
  OT_h = acc[o:o+64] * (1/Z)        (DVE approx-reciprocal + gpsimd partition
                                     broadcast + DVE multiply), bf16
  y    [2048, 384]  = sum_h OT_h^T @ Wproj_h   (partial; f32 out)
"""

import os

os.environ.pop("JAX_PLATFORMS", None)  # the bass PJRT path needs the axon platform

import numpy as np
import ml_dtypes

import concourse.mybir as mybir
import concourse.tile as tile
from concourse import bacc
from concourse.bass_utils import run_bass_kernel_spmd

BF16 = ml_dtypes.bfloat16

# problem shapes (hardcoded per contract)
B, N, D = 4, 2048, 384
H, DH = 8, 48
SCALE = DH**-0.5
N_CORES = 8
HP = 4  # heads per core
DHP = 64  # padded head dim
P = 128
NKC = N // P  # 16 key-row chunks
ZOFF = 32  # partition offset of the fused softmax-denominator (Z) row within a
# head's 64-row block: engines need 32-aligned partition starts, so the ones
# column sits at col 32 of each head's V' block; v-dims occupy cols
# [0,32) and [33,49), the rest are zero. Wproj rows are laid out to match,
# with zeros at the Z/pad rows.

LAST_EXEC_NS = None
_CACHE = {}


def _build_bass():
    f32 = mybir.dt.float32
    bf16 = mybir.dt.bfloat16
    EXP = mybir.ActivationFunctionType.Exp

    nc = bacc.Bacc("TRN2", target_bir_lowering=False, debug=False, num_devices=N_CORES)
    xbT = nc.dram_tensor("xbT", [D, N], bf16, kind="ExternalInput").ap()
    wq = nc.dram_tensor("wq", [D, HP * DHP], bf16, kind="ExternalInput").ap()
    wk = nc.dram_tensor("wk", [D, HP * DHP], bf16, kind="ExternalInput").ap()
    wv = nc.dram_tensor("wv", [D, HP * DHP], bf16, kind="ExternalInput").ap()
    wpj = nc.dram_tensor("wpj", [2, P, D], bf16, kind="ExternalInput").ap()
    y = nc.dram_tensor("y", [N, D], f32, kind="ExternalOutput").ap()

    with tile.TileContext(nc) as tc:
        with (
            tc.tile_pool(name="const", bufs=1) as cpool,
            tc.tile_pool(name="epool", bufs=6) as epool,
            tc.tile_pool(name="rpool", bufs=3) as rpool,
            tc.tile_pool(name="ysb", bufs=6) as ypool,
            tc.tile_pool(name="simps", bufs=2, space="PSUM") as simps,
            tc.tile_pool(name="accps", bufs=2, space="PSUM") as accps,
        ):
            # ---- load weights / x ----
            # Early-DMA queue plan: weights on the gpsimd SWDGE queue (K first
            # - the first matmul groups need it), xT halves alternating across
            # the two HWDGE queues so the first qkv groups unblock early.
            wq_sb, wk_sb, wv_sb = [], [], []
            for name, srct, dst in (("wk", wk, wk_sb), ("wq", wq, wq_sb), ("wv", wv, wv_sb)):
                for i in range(3):
                    t = cpool.tile([P, HP * DHP], bf16, name=f"{name}{i}", tag=f"{name}{i}")
                    nc.gpsimd.dma_start(out=t[:], in_=srct[i * P : (i + 1) * P, :])
                    dst.append(t)
            xT = [cpool.tile([P, N], bf16, name=f"xT{i}", tag=f"xT{i}") for i in range(3)]
            for hf in range(2):  # halves so the first qkv groups unblock early
                for i in range(3):
                    eng = nc.sync if (i + hf) % 2 == 0 else nc.scalar
                    eng.dma_start(
                        out=xT[i][:, hf * (N // 2) : (hf + 1) * (N // 2)],
                        in_=xbT[i * P : (i + 1) * P, hf * (N // 2) : (hf + 1) * (N // 2)],
                    )
            wpj_sb = []
            for p in range(2):
                t = cpool.tile([P, D], bf16, name=f"wpj{p}", tag=f"wpj{p}")
                nc.gpsimd.dma_start(out=t[:], in_=wpj[p])
                wpj_sb.append(t)

            # ---- QKV projection ----
            QT = [cpool.tile([P, N], bf16, name=f"QT{p}", tag=f"QT{p}") for p in range(2)]
            KT = [cpool.tile([P, N], bf16, name=f"KT{p}", tag=f"KT{p}") for p in range(2)]

            def qk_pair(p):
                for gi, (w_sb, dstl) in enumerate(((wk_sb, KT), (wq_sb, QT))):
                    for j in range(4):
                        pool = simps if (gi * 4 + j) % 2 == 0 else accps
                        ps = pool.tile([P, 512], f32, name="qkvps", tag="sim" if pool is simps else "acc")
                        for dk in range(3):
                            nc.tensor.matmul(
                                ps[:],
                                lhsT=w_sb[dk][:, p * P : (p + 1) * P],
                                rhs=xT[dk][:, j * 512 : (j + 1) * 512],
                                start=(dk == 0),
                                stop=(dk == 2),
                            )
                        if j % 2 == 0:
                            nc.vector.tensor_copy(dstl[p][:, j * 512 : (j + 1) * 512], ps[:])
                        else:
                            nc.scalar.copy(dstl[p][:, j * 512 : (j + 1) * 512], ps[:])

            qk_pair(0)
            qk_pair(1)
            V = [cpool.tile([P, HP * DHP], bf16, name=f"V{i}", tag=f"V{i}") for i in range(NKC)]
            def v_chunks(i0, i1):
             if True:
              for i in range(i0, i1):
                pool = simps if i % 2 == 0 else accps
                ps = pool.tile([P, HP * DHP], f32, name="vps", tag="sim" if pool is simps else "acc")
                for dk in range(3):
                    nc.tensor.matmul(
                        ps[:],
                        lhsT=xT[dk][:, i * P : (i + 1) * P],
                        rhs=wv_sb[dk][:],
                        start=(dk == 0),
                        stop=(dk == 2),
                    )
                t = V[i]
                if i % 2 == 0:
                    nc.vector.tensor_copy(t[:], ps[:])
                else:
                    nc.scalar.copy(t[:], ps[:])
                # ones (Z) column of each head block, at col h*64+ZOFF
                zcols = t[:].rearrange("p (h c) -> p h c", c=DHP)[:, :, ZOFF : ZOFF + 1]
                nc.gpsimd.memset(zcols, 1.0)
            v_chunks(0, NKC)

            # ---- attention ----
            OT = [cpool.tile([P, N], bf16, name=f"OT{p}", tag=f"OT{p}") for p in range(2)]
            NQH = N // 1024  # q halves
            def attention_block(p, qh, fine=False):
                    acc = accps.tile([P, 1024], f32, name="acc", tag="acc")
                    for hh in range(2):
                        o = hh * DHP
                        h = p * 2 + hh
                        for kc in range(NKC):
                            sp = simps.tile([P, 1024], f32, name="sim", tag="sim")
                            for j in range(2):
                                nc.tensor.matmul(
                                    sp[:, j * 512 : (j + 1) * 512],
                                    lhsT=KT[p][o : o + DHP, kc * P : (kc + 1) * P],
                                    rhs=QT[p][
                                        o : o + DHP,
                                        qh * 1024 + j * 512 : qh * 1024 + (j + 1) * 512,
                                    ],
                                    start=True,
                                    stop=True,
                                )
                            e = epool.tile([P, 1024], bf16, name="E", tag="E")
                            nc.scalar.activation(e[:], sp[:], EXP)
                            for j in range(2):
                                nc.tensor.matmul(
                                    acc[o : o + DHP, j * 512 : (j + 1) * 512],
                                    lhsT=V[kc][:, h * DHP : (h + 1) * DHP],
                                    rhs=e[:, j * 512 : (j + 1) * 512],
                                    start=(kc == 0),
                                    stop=(kc == NKC - 1),
                                )
                    ns = 2 if fine else 1  # last block: finer pieces unblock proj sooner
                    w = 1024 // ns
                    for hh in range(2):
                        o = hh * DHP
                        for s in range(ns):
                            zrow = rpool.tile([1, w], f32, name="zrow", tag="zrow")
                            nc.vector.tensor_copy(
                                zrow[:], acc[o + ZOFF : o + ZOFF + 1, s * w : (s + 1) * w]
                            )
                            r = rpool.tile([1, w], f32, name="r", tag="r")
                            nc.vector.reciprocal_approx_fast(r[:], zrow[:])
                            R = rpool.tile([DHP, w], f32, name="R", tag="R")
                            nc.gpsimd.partition_broadcast(R[:], r[:], channels=DHP)
                            nc.vector.tensor_mul(
                                OT[p][o : o + DHP, qh * 1024 + s * w : qh * 1024 + (s + 1) * w],
                                acc[o : o + DHP, s * w : (s + 1) * w],
                                R[:],
                            )

            # ---- output projection (partial: this core's 4 heads) ----
            def proj_block(mc0, mc1):
                for mc in range(mc0, mc1):
                    pool = simps if mc % 2 == 0 else accps
                    yp = pool.tile([P, D], f32, name="yp", tag="sim" if pool is simps else "acc")
                    for p in range(2):  # K=128 covers both heads of the pair
                        nc.tensor.matmul(
                            yp[:],
                            lhsT=OT[p][:, mc * P : (mc + 1) * P],
                            rhs=wpj_sb[p][:],
                            start=(p == 0),
                            stop=(p == 1),
                        )
                    ys = ypool.tile([P, D], f32, name="ys", tag="ys")
                    if mc % 2 == 0:
                        nc.vector.tensor_copy(ys[:], yp[:])
                        nc.sync.dma_start(out=y[mc * P : (mc + 1) * P, :], in_=ys[:])
                    else:
                        nc.scalar.copy(ys[:], yp[:])
                        nc.gpsimd.dma_start(out=y[mc * P : (mc + 1) * P, :], in_=ys[:])

            attention_block(0, 0)
            attention_block(1, 0)
            attention_block(0, 1)
            attention_block(1, 1)
            proj_block(0, NKC)

    nc.compile()
    return nc


def _prep_core_inputs(x, Wqkv, Wproj, core):
    b, hg = core // 2, core % 2
    heads = [hg * HP + i for i in range(HP)]
    xbT = np.ascontiguousarray(x[b].astype(BF16).T)
    wq = np.zeros((D, HP * DHP), np.float32)
    wk = np.zeros((D, HP * DHP), np.float32)
    wv = np.zeros((D, HP * DHP), np.float32)
    wpj = np.zeros((2, P, D), np.float32)
    for i, h in enumerate(heads):
        wq[:, i * DHP : i * DHP + DH] = Wqkv[:, h * DH : (h + 1) * DH] * SCALE
        wk[:, i * DHP : i * DHP + DH] = Wqkv[:, H * DH + h * DH : H * DH + (h + 1) * DH]
        wv_h = Wqkv[:, 2 * H * DH + h * DH : 2 * H * DH + (h + 1) * DH]
        wpj_h = Wproj[h * DH : (h + 1) * DH, :]
        # v-dims at cols [0,ZOFF) and [ZOFF+1, DH+1); ones (Z) column at ZOFF
        wv[:, i * DHP : i * DHP + ZOFF] = wv_h[:, :ZOFF]
        wv[:, i * DHP + ZOFF + 1 : i * DHP + DH + 1] = wv_h[:, ZOFF:]
        o = (i % 2) * DHP
        wpj[i // 2, o : o + ZOFF, :] = wpj_h[:ZOFF, :]
        wpj[i // 2, o + ZOFF + 1 : o + DH + 1, :] = wpj_h[ZOFF:, :]
    return {
        "xbT": xbT,
        "wq": wq.astype(BF16),
        "wk": wk.astype(BF16),
        "wv": wv.astype(BF16),
        "wpj": wpj.astype(BF16),
    }


def kernel(x, Wqkv, Wproj, bproj):
    global LAST_EXEC_NS
    if "nc" not in _CACHE:
        _CACHE["nc"] = _build_bass()
    nc = _CACHE["nc"]
    in_maps = [_prep_core_inputs(x, Wqkv, Wproj, c) for c in range(N_CORES)]
    try:
        res = run_bass_kernel_spmd(nc, in_maps, core_ids=list(range(N_CORES)))
    except Exception:
        res = run_bass_kernel_spmd(nc, in_maps, core_ids=list(range(N_CORES)))
    LAST_EXEC_NS = res.exec_time_ns
    out = np.empty((B, N, D), np.float32)
    for b in range(B):
        out[b] = res.results[2 * b]["y"] + res.results[2 * b + 1]["y"]
    out += bproj.astype(np.float32)[None, None, :]
    return out

